# revision 8
# baseline (speedup 1.0000x reference)
"""GAT-style attention kernel for Trainium2, 8 NeuronCores.

Reference computation (N=M=8192, D=256, f32):
    e1 = input1 @ a1; e2 = (input2 @ a2).T
    e  = leaky_relu(e1 + e2, 0.2)
    att = softmax(where(adj>0, e, -9e15), axis=1)
    att = att * adj.sum(1, keepdims=True)
    att = att*0.5 + adj*0.5
    out = att @ input2

Device math per row i (w_ij = exp(leaky_relu(e1_i + e2_j))):
    denom_i = sum_j adj_ij w_ij ; deg_i = sum_j adj_ij ; delta_i = deg_i/denom_i
    out_i = 0.5 * [ (delta_i * (adj.w) + adj) @ input2 ]_i

Sharding: rows of N across 8 cores (1024 each); input2/a1/a2 replicated.

Per-core pipeline, 8 blocks of (128 rows x 8192 cols):
    adjb = bf16(adj block)                     [SWDGE cast-DMA]
    m    = adjb - 1, accum+8192 -> deg         [DVE TS 4x + accum]
    t    = m + e2b', e2b' = e2/BIG (bf16)      [DVE TT 2x, in-place on m]
    lr   = prelu(BIG*t + e1_i, 0.2)            [ACT f32; BIG*(adj-1) masks]
    num  = exp(lr), accum -> denom             [ACT, bf16 out]
    s    = num * delta_i                       [DVE TS 4x, in-place]
    att  = s + adjb                            [Pool TT, in-place on s]
    attT chunks via PE transpose (exact)       [PE, PSUM bf16]
    att_sb = copy(attT)                        [DVE PSUM->SBUF 2x]
    acc += att_sb_chunk.T @ input2_bf16_chunk  [PE matmul]
    out = 0.5 * acc                            [DVE]
"""

import os
import numpy as np

import concourse.bass as bass
import concourse.bacc as bacc
import concourse.tile as tile
from concourse import mybir
from concourse.bass_utils import run_bass_kernel_spmd

try:
    import ml_dtypes

    _BF16_NP = ml_dtypes.bfloat16
except Exception:  # pragma: no cover
    _BF16_NP = None

N, M, D = 8192, 8192, 256
NCORES = 8
ROWS = N // NCORES
P = 128
NBLK = ROWS // P  # 8
NCHUNK = M // P  # 64
BIG = 150.0
SLOPE = 0.2
GRP = 8  # transpose chunks per PSUM staging bank

F32 = mybir.dt.float32
BF16 = mybir.dt.bfloat16

LAST_EXEC_NS = None
_CACHED = None


def _build_kernel():
    nc = bacc.Bacc("TRN2", target_bir_lowering=False, debug=False)

    inp1 = nc.dram_tensor("input1", [ROWS, D], F32, kind="ExternalInput").ap()
    inp2 = nc.dram_tensor("input2", [M, D], F32, kind="ExternalInput").ap()
    adj = nc.dram_tensor("adj", [ROWS, M], F32, kind="ExternalInput").ap()
    a1b = nc.dram_tensor("a1b", [P, D], F32, kind="ExternalInput").ap()
    a2b = nc.dram_tensor("a2b", [P, D], F32, kind="ExternalInput").ap()
    identd = nc.dram_tensor("identd", [P, P], BF16, kind="ExternalInput").ap()
    out = nc.dram_tensor("out", [ROWS, D], F32, kind="ExternalOutput").ap()

    # DRAM bounce for flattening e2 (computed column-wise) into row order
    e2d = nc.dram_tensor("e2d", [1, M], F32).ap()

    AL = mybir.AluOpType

    with tile.TileContext(nc) as tc:
        with (
            tc.tile_pool(name="persist", bufs=1) as persist,
            tc.tile_pool(name="setup", bufs=2) as setup,
            tc.tile_pool(name="small", bufs=4) as small,
            tc.tile_pool(name="adjp", bufs=2) as adjp,
            tc.tile_pool(name="mtp", bufs=2) as mtp,
            tc.tile_pool(name="lrp", bufs=1) as lrp,
            tc.tile_pool(name="nump", bufs=2) as nump,
            tc.tile_pool(name="attp", bufs=3) as attp,
            tc.tile_pool(name="outp", bufs=2) as outp,
            tc.tile_pool(name="psA", bufs=3, space="PSUM") as psA,
            tc.tile_pool(name="psO", bufs=2, space="PSUM") as psO,
        ):
            # ---------------- setup ----------------
            ident = persist.tile([P, P], BF16)
            nc.sync.dma_start(ident[:], identd[:])
            a1t = persist.tile([P, D], F32)
            nc.sync.dma_start(a1t[:], a1b[:])
            a2t = persist.tile([P, D], F32)
            nc.sync.dma_start(a2t[:], a2b[:])

            inp2b = persist.tile([P, NCHUNK * D], BF16)
            e2col = persist.tile([P, NCHUNK], F32)
            e1col = persist.tile([P, NBLK], F32)
            for t in range(NCHUNK):
                tmp = setup.tile([P, D], F32, tag="i2tile")
                nc.sync.dma_start(tmp[:], inp2[t * P : (t + 1) * P, :])
                nc.vector.tensor_copy(inp2b[:, t * D : (t + 1) * D], tmp[:])
                scr = setup.tile([P, D], F32, tag="scratch")
                nc.vector.affine_mul_reduce(
                    out=scr[:],
                    accum_out=e2col[:, t : t + 1],
                    in0=tmp[:],
                    in1=a2t[:],
                    scale=1.0,
                    bias=0.0,
                )
            for b in range(NBLK):
                tmp = setup.tile([P, D], F32, tag="i1tile")
                nc.sync.dma_start(tmp[:], inp1[b * P : (b + 1) * P, :])
                scr = setup.tile([P, D], F32, tag="scratch")
                nc.vector.affine_mul_reduce(
                    out=scr[:],
                    accum_out=e1col[:, b : b + 1],
                    in0=tmp[:],
                    in1=a1t[:],
                    scale=1.0,
                    bias=0.0,
                )

            # scale e2 by 1/BIG for the masked-prelu trick (t kept in bf16,
            # prelu applies scale=BIG to restore)
            e2cs = persist.tile([P, NCHUNK], F32)
            nc.vector.tensor_scalar(e2cs[:], e2col[:], 1.0 / BIG, None, AL.mult)

            # e2cs[p, t] = e2[t*128+p]/BIG  ->  e2d[j] (scatter via stride AP)
            e2d_scat = e2d.rearrange("one (t p) -> one p t", p=P, t=NCHUNK)
            nc.sync.dma_start(e2d_scat[0], e2cs[:])

            # e2b' = broadcast(e2/BIG) to all partitions, cast f32->bf16
            e2b = persist.tile([P, M], BF16)
            nc.gpsimd.dma_start(e2b[:], e2d[:].broadcast_to([P, M]))

            # ---------------- main loop ----------------
            for b in range(NBLK):
                adjb = adjp.tile([P, M], BF16)
                nc.gpsimd.dma_start(adjb[:], adj[b * P : (b + 1) * P, :])

                # m = adj - 1 ; deg = sum(m) + 8192
                mt = mtp.tile([P, M], BF16)
                deg = small.tile([P, 1], F32, tag="deg")
                nc.vector.tensor_scalar(
                    mt[:], adjb[:], -1.0, float(M), AL.add, AL.add, accum_out=deg[:]
                )
                # t = m + e2b'  (in place on m)
                nc.vector.tensor_tensor(mt[:], mt[:], e2b[:], AL.add)
                # lr = prelu(BIG*t + e1_b, 0.2)
                lr = lrp.tile([P, M], F32)
                nc.scalar.activation(
                    lr[:],
                    mt[:],
                    mybir.ActivationFunctionType.Prelu,
                    bias=e1col[:, b : b + 1],
                    scale=BIG,
                    alpha=SLOPE,
                )
                # num = exp(lr), accum -> denom
                num = nump.tile([P, M], BF16)
                denom = small.tile([P, 1], F32, tag="denom")
                nc.scalar.activation(
                    num[:],
                    lr[:],
                    mybir.ActivationFunctionType.Exp,
                    accum_out=denom[:],
                )

                # delta = deg / denom
                rec = small.tile([P, 1], F32, tag="rec")
                nc.vector.reciprocal(rec[:], denom[:])
                delta = small.tile([P, 1], F32, tag="delta")
                nc.vector.tensor_tensor(delta[:], deg[:], rec[:], AL.mult)

                # s = num * delta (in place); att = s + adjb (in place, Pool)
                nc.vector.tensor_scalar(num[:], num[:], delta[:], None, AL.mult)
                nc.gpsimd.tensor_tensor(num[:], num[:], adjb[:], AL.add)

                # PE transpose att chunks -> PSUM, copy to SBUF, matmul
                acc = psO.tile([P, D], F32)
                for g in range(NCHUNK // GRP):
                    stage = psA.tile([P, GRP * P], BF16)
                    for k in range(GRP):
                        c = g * GRP + k
                        nc.tensor.matmul(
                            stage[:, k * P : (k + 1) * P],
                            num[:, c * P : (c + 1) * P],
                            ident[:],
                            is_transpose=True,
                            start=True,
                            stop=True,
                        )
                    att = attp.tile([P, GRP * P], BF16)
                    nc.vector.tensor_copy(att[:], stage[:])
                    for k in range(GRP):
                        c = g * GRP + k
                        nc.tensor.matmul(
                            acc[:],
                            att[:, k * P : (k + 1) * P],
                            inp2b[:, c * D : (c + 1) * D],
                            start=(c == 0),
                            stop=(c == NCHUNK - 1),
                        )

                ot = outp.tile([P, D], F32)
                nc.vector.tensor_scalar(ot[:], acc[:], 0.5, None, AL.mult)
                nc.sync.dma_start(out[b * P : (b + 1) * P, :], ot[:])

    nc.compile()
    return nc


def _get_nc():
    global _CACHED
    if _CACHED is None:
        _CACHED = _build_kernel()
    return _CACHED


def kernel(input1, input2, adj, a1, a2):
    global LAST_EXEC_NS
    nc = _get_nc()

    a1bv = np.ascontiguousarray(np.broadcast_to(np.asarray(a1, np.float32).reshape(1, D), (P, D)))
    a2bv = np.ascontiguousarray(np.broadcast_to(np.asarray(a2, np.float32).reshape(1, D), (P, D)))
    ident = np.eye(P, dtype=_BF16_NP)

    input1 = np.ascontiguousarray(input1, dtype=np.float32)
    input2 = np.ascontiguousarray(input2, dtype=np.float32)
    adj = np.ascontiguousarray(adj, dtype=np.float32)

    in_maps = []
    for c in range(NCORES):
        r0, r1 = c * ROWS, (c + 1) * ROWS
        in_maps.append(
            {
                "input1": input1[r0:r1],
                "input2": input2,
                "adj": adj[r0:r1],
                "a1b": a1bv,
                "a2b": a2bv,
                "identd": ident,
            }
        )

    trace = bool(os.environ.get("GAT_TRACE"))
    res = run_bass_kernel_spmd(nc, in_maps, core_ids=list(range(NCORES)), trace=trace)
    LAST_EXEC_NS = res.exec_time_ns
    outs = [res.results[c]["out"] for c in range(NCORES)]
    return np.concatenate(outs, axis=0).astype(np.float32)


# revision 14
# speedup vs baseline: 1.6186x; 1.6186x over previous
"""GAT-style attention kernel for Trainium2, 8 NeuronCores.

Reference computation (N=M=8192, D=256, f32):
    e1 = input1 @ a1; e2 = (input2 @ a2).T
    e  = leaky_relu(e1 + e2, 0.2)
    att = softmax(where(adj>0, e, -9e15), axis=1)
    att = att * adj.sum(1, keepdims=True)
    att = att*0.5 + adj*0.5
    out = att @ input2

Device math per row i (w_ij = exp(leaky_relu(e1_i + e2_j))):
    denom_i = sum_j adj_ij w_ij ; deg_i = sum_j adj_ij ; delta_i = deg_i/denom_i
    out_i = 0.5 * [ (delta_i * (adj.w) + adj) @ input2 ]_i

Sharding: rows of N across 8 cores (1024 each); input2/a1/a2 replicated.

Per-core pipeline, 8 blocks of (128 rows x 8192 cols):
    adjb = bf16(adj block)                     [SWDGE cast-DMA]
    m    = adjb - 1, accum+8192 -> deg         [DVE TS 4x + accum]
    t    = m + e2b', e2b' = e2/BIG (bf16)      [DVE TT 2x, in-place on m]
    lr   = prelu(BIG*t + e1_i, 0.2)            [ACT f32; BIG*(adj-1) masks]
    num  = exp(lr), accum -> denom             [ACT, bf16 out]
    s    = num * delta_i                       [DVE TS 4x, in-place]
    att  = s + adjb                            [Pool TT, in-place on s]
    attT chunks via PE transpose (exact)       [PE, PSUM bf16]
    att_sb = copy(attT)                        [DVE PSUM->SBUF 2x]
    acc += att_sb_chunk.T @ input2_bf16_chunk  [PE matmul]
    out = 0.5 * acc                            [DVE]
"""

import os
import numpy as np

import concourse.bass as bass
import concourse.bacc as bacc
import concourse.tile as tile
from concourse import mybir
from concourse.bass_utils import run_bass_kernel_spmd

try:
    import ml_dtypes

    _BF16_NP = ml_dtypes.bfloat16
except Exception:  # pragma: no cover
    _BF16_NP = None

N, M, D = 8192, 8192, 256
NCORES = 8
ROWS = N // NCORES
P = 128
NBLK = ROWS // P  # 8
NCHUNK = M // P  # 64
BIG = 150.0
SLOPE = 0.2
GRP = 8  # transpose chunks per PSUM staging bank

F32 = mybir.dt.float32
BF16 = mybir.dt.bfloat16

LAST_EXEC_NS = None
_CACHED = None


def _build_kernel():
    nc = bacc.Bacc("TRN2", target_bir_lowering=False, debug=False)

    inp1 = nc.dram_tensor("input1", [ROWS, D], F32, kind="ExternalInput").ap()
    inp2 = nc.dram_tensor("input2", [M, D], F32, kind="ExternalInput").ap()
    adj = nc.dram_tensor("adj", [ROWS, M], F32, kind="ExternalInput").ap()
    a1b = nc.dram_tensor("a1b", [P, D], F32, kind="ExternalInput").ap()
    a2b = nc.dram_tensor("a2b", [P, D], F32, kind="ExternalInput").ap()
    identd = nc.dram_tensor("identd", [P, P], BF16, kind="ExternalInput").ap()
    out = nc.dram_tensor("out", [ROWS, D], F32, kind="ExternalOutput").ap()

    # DRAM bounce for flattening e2 (computed column-wise) into row order
    e2d = nc.dram_tensor("e2d", [1, M], F32).ap()

    AL = mybir.AluOpType

    with tile.TileContext(nc) as tc:
        with (
            tc.tile_pool(name="persist", bufs=1) as persist,
            tc.tile_pool(name="setup", bufs=2) as setup,
            tc.tile_pool(name="small", bufs=4) as small,
            tc.tile_pool(name="adjp", bufs=2) as adjp,
            tc.tile_pool(name="mtp", bufs=2) as mtp,
            tc.tile_pool(name="lrp", bufs=1) as lrp,
            tc.tile_pool(name="nump", bufs=2) as nump,
            tc.tile_pool(name="attp", bufs=2) as attp,
            tc.tile_pool(name="outp", bufs=2) as outp,
            tc.tile_pool(name="psA", bufs=3, space="PSUM") as psA,
            tc.tile_pool(name="psO", bufs=2, space="PSUM") as psO,
        ):
            # ---------------- setup ----------------
            ident = persist.tile([P, P], BF16)
            nc.sync.dma_start(ident[:], identd[:])
            a1t = persist.tile([P, D], F32)
            nc.sync.dma_start(a1t[:], a1b[:])
            a2t = persist.tile([P, D], F32)
            nc.sync.dma_start(a2t[:], a2b[:])

            inp2b = persist.tile([P, NCHUNK * D], BF16)
            e2col = persist.tile([P, NCHUNK], F32)
            e1col = persist.tile([P, NBLK], F32)
            for t in range(NCHUNK):
                tmp = setup.tile([P, D], F32, tag="itile")
                nc.sync.dma_start(tmp[:], inp2[t * P : (t + 1) * P, :])
                nc.vector.tensor_copy(inp2b[:, t * D : (t + 1) * D], tmp[:])
                scr = setup.tile([P, D], F32, tag="scratch")
                nc.vector.affine_mul_reduce(
                    out=scr[:],
                    accum_out=e2col[:, t : t + 1],
                    in0=tmp[:],
                    in1=a2t[:],
                    scale=1.0,
                    bias=0.0,
                )
            for b in range(NBLK):
                tmp = setup.tile([P, D], F32, tag="itile")
                nc.sync.dma_start(tmp[:], inp1[b * P : (b + 1) * P, :])
                scr = setup.tile([P, D], F32, tag="scratch")
                nc.vector.affine_mul_reduce(
                    out=scr[:],
                    accum_out=e1col[:, b : b + 1],
                    in0=tmp[:],
                    in1=a1t[:],
                    scale=1.0,
                    bias=0.0,
                )

            # scale e2 by 1/BIG for the masked-prelu trick (t kept in bf16,
            # prelu applies scale=BIG to restore)
            e2cs = persist.tile([P, NCHUNK], F32)
            nc.vector.tensor_scalar(e2cs[:], e2col[:], 1.0 / BIG, None, AL.mult)

            # e2cs[p, t] = e2[t*128+p]/BIG  ->  e2d[j] (scatter via stride AP)
            e2d_scat = e2d.rearrange("one (t p) -> one p t", p=P, t=NCHUNK)
            nc.sync.dma_start(e2d_scat[0], e2cs[:])

            # e2b' = broadcast(e2/BIG) to all partitions, kept in f32 so the
            # only bf16 rounding of e2 happens once (in t)
            e2b = persist.tile([P, M], F32)
            nc.sync.dma_start(e2b[:], e2d[:].broadcast_to([P, M]))

            # ---------------- main loop ----------------
            for b in range(NBLK):
                adjb = adjp.tile([P, M], BF16)
                nc.gpsimd.dma_start(adjb[:], adj[b * P : (b + 1) * P, :])

                # m = adj - 1 ; deg = sum(m) + 8192
                mt = mtp.tile([P, M], BF16)
                deg = small.tile([P, 1], F32, tag="deg")
                nc.vector.tensor_scalar(
                    mt[:], adjb[:], -1.0, float(M), AL.add, AL.add, accum_out=deg[:]
                )
                # t = m + e2b'  (in place on m; Pool engine, f32 rhs)
                nc.gpsimd.tensor_tensor(mt[:], mt[:], e2b[:], AL.add)
                # lr = prelu(BIG*t + e1_b, 0.2)
                lr = lrp.tile([P, M], F32)
                nc.scalar.activation(
                    lr[:],
                    mt[:],
                    mybir.ActivationFunctionType.Prelu,
                    bias=e1col[:, b : b + 1],
                    scale=BIG,
                    alpha=SLOPE,
                )
                # num = exp(lr), accum -> denom
                num = nump.tile([P, M], BF16)
                denom = small.tile([P, 1], F32, tag="denom")
                nc.scalar.activation(
                    num[:],
                    lr[:],
                    mybir.ActivationFunctionType.Exp,
                    accum_out=denom[:],
                )

                # delta = deg / denom
                rec = small.tile([P, 1], F32, tag="rec")
                nc.vector.reciprocal(rec[:], denom[:])
                delta = small.tile([P, 1], F32, tag="delta")
                nc.vector.tensor_tensor(delta[:], deg[:], rec[:], AL.mult)

                # s = num * delta (in place); att = s + adjb (in place)
                nc.vector.tensor_scalar(num[:], num[:], delta[:], None, AL.mult)
                nc.vector.tensor_tensor(num[:], num[:], adjb[:], AL.add)

                # PE transpose att chunks -> PSUM, copy to SBUF, matmul
                acc = psO.tile([P, D], F32)
                for g in range(NCHUNK // GRP):
                    stage = psA.tile([P, GRP * P], BF16)
                    for k in range(GRP):
                        c = g * GRP + k
                        nc.tensor.matmul(
                            stage[:, k * P : (k + 1) * P],
                            num[:, c * P : (c + 1) * P],
                            ident[:],
                            is_transpose=True,
                            start=True,
                            stop=True,
                        )
                    att = attp.tile([P, GRP * P], BF16)
                    nc.vector.tensor_copy(att[:], stage[:])
                    for k in range(GRP):
                        c = g * GRP + k
                        nc.tensor.matmul(
                            acc[:],
                            att[:, k * P : (k + 1) * P],
                            inp2b[:, c * D : (c + 1) * D],
                            start=(c == 0),
                            stop=(c == NCHUNK - 1),
                        )

                ot = outp.tile([P, D], F32)
                nc.vector.tensor_scalar(ot[:], acc[:], 0.5, None, AL.mult)
                nc.sync.dma_start(out[b * P : (b + 1) * P, :], ot[:])

    nc.compile()
    return nc


def _get_nc():
    global _CACHED
    if _CACHED is None:
        _CACHED = _build_kernel()
    return _CACHED


def kernel(input1, input2, adj, a1, a2):
    global LAST_EXEC_NS
    nc = _get_nc()

    a1bv = np.ascontiguousarray(np.broadcast_to(np.asarray(a1, np.float32).reshape(1, D), (P, D)))
    a2bv = np.ascontiguousarray(np.broadcast_to(np.asarray(a2, np.float32).reshape(1, D), (P, D)))
    ident = np.eye(P, dtype=_BF16_NP)

    input1 = np.ascontiguousarray(input1, dtype=np.float32)
    input2 = np.ascontiguousarray(input2, dtype=np.float32)
    adj = np.ascontiguousarray(adj, dtype=np.float32)

    in_maps = []
    for c in range(NCORES):
        r0, r1 = c * ROWS, (c + 1) * ROWS
        in_maps.append(
            {
                "input1": input1[r0:r1],
                "input2": input2,
                "adj": adj[r0:r1],
                "a1b": a1bv,
                "a2b": a2bv,
                "identd": ident,
            }
        )

    trace = bool(os.environ.get("GAT_TRACE"))
    res = run_bass_kernel_spmd(nc, in_maps, core_ids=list(range(NCORES)), trace=trace)
    LAST_EXEC_NS = res.exec_time_ns
    outs = [res.results[c]["out"] for c in range(NCORES)]
    return np.concatenate(outs, axis=0).astype(np.float32)
